# revision 2
# baseline (speedup 1.0000x reference)
"""MixIT loss kernel for Trainium2 (raw Bass), 8-way data-parallel over batch.

The loss depends only on the 10x10 Gram G of D = [sources(8); mixtures(2)]
over T=32000:
  d1_k = sum_m a_km (y_m - 2*C1_m) + (1+tau)E1
  d0_k = sum_m a_km (y_m + 2*C0_m) - 2*sum_m y_m + [(1+tau)E0 + 2*sum(C0) + sum(G8)]
  y = G8 @ a_k,  per_sample = (10/ln10) * (ln(min_k d1*d0) - ln(E0*E1))
(C0/C1 = <s_m, x0/1>, E = ||x||^2, G8 = source Gram, a_k = combo k's
mixture-1 membership; K = 254 combos.)

Per core (one batch sample; host averages 8 scalars and applies the constant
10/ln10 scale):
  1. Host pre-interleaves the sample to R[p, b*100 + s*10 + i] =
     D[s, p*250 + b*10 + i]; two DMA waves ([17, 8] Gram blocks) load it.
  2. Each wave is chased on the same HWDGE ring by a tiny SBUF->SBUF
     "sentinel" DMA carrying the wave's semaphore: per-engine FIFO order
     makes its 16 incs imply the wave landed, ~1us earlier than the data
     DMA's own HBM-receipt-bound semaphore.
  3. DVE (and GpSimd, for one chunk) cast f32 -> bf16 in 5 chunks; PE runs
     25 bf16 [128,100]^2 Gram matmuls accumulating a 100x100 PSUM whose
     i-aligned sub-grid holds partial Grams.
  4. The fold contracts with a block-diagonal selector SELDIAG so its PSUM
     output qz10 = [G8 rows; -2*C1 row; 2*C0 row] - directly the (bf16)
     lhsT of the combined qt4 matmuls producing [y-2C1 | y+2C0].
  5. ne row accumulates colsum(A .* qt4) and the -2*sum(y) correction via
     three small matmuls; DVE folds the two scalar constants while taking
     the product of the halves; min over 254; ACT computes ln(min/(E0*E1))
     with a scale-AP and issues the 4-byte output DMA itself.
  6. Scalar constants come from 3 gather matmuls (constant selector columns
     against qz10) + an ACT copy whose accum_out yields sum(G8)+2*sum(C0)
     for free; engines require equal partition bases on all operands, so all
     partition movement goes through PE contractions.
  7. No nc.Block(), no wait on the output DMA: the compiler's fixed
     semaphore-reset postamble (~7us, behind its own global barrier) gives
     the store ample time to land before NEFF completion.

Raw Bass: every same-engine RAW chain needs an explicit semaphore handshake
(engines are deep pipelines); cross-engine deps use standalone wait_ge.
"""

import itertools
from contextlib import ExitStack

import numpy as np

from concourse import bass, mybir
from concourse.bass_utils import run_bass_kernel_spmd

F32 = mybir.dt.float32
BF16 = mybir.dt.bfloat16

B = 8
M = 8
NMIX = 2
NSIG = M + NMIX  # 10
T = 32000
P = 128
NCHUNK = T // P  # 250
LBLK = 10
NBLK = NCHUNK // LBLK  # 25
BW = NSIG * LBLK  # 100
K = 2**M - 2  # 254
TAU = 1e-6
LOG10_SCALE = 10.0 / float(np.log(10.0))

WAVE_EDGES = [0, 17, 25]  # gram-block ranges per DMA wave
N_WAVES = len(WAVE_EDGES) - 1
# cast chunks (block ranges); chunk 3 runs on GpSimd, rest on DVE
CAST_EDGES = [0, 6, 12, 17, 21, 25]

C2W = 768  # cst2 width (f32), cast to bf16 in-kernel
RHS4_C = 0          # [10, 508]: rows 0-7 [A|A]; row 8 [1|0]; row 9 [0|1]
PA2_C = 508         # [9, 254]: rows 0-7 A, row 8 = -2 (prod4b9 multiplier)
ONES8_C = 762       # [9, 1] ones column
WZ_C = 764          # [10,1] u-gather weights [1x8, 0, 1]
HE9_C = 765         # [10,1] 0.5*e9
MHE8_C = 766        # [10,1] -0.5*e8


def _assignment_matrix() -> np.ndarray:
    cols = [a for a in itertools.product([0, 1], repeat=M) if 0 < sum(a) < M]
    return np.array(cols, dtype=np.float32).T.copy()


def _sel_diag() -> np.ndarray:
    """[100, 100] fold selector: SEL[q,c]=delta for c<8; col8=-2*e9; col9=2*e8.

    Columns are signal-major (col = s*LBLK + i): fold mm i uses lhsT cols
    [10i:10i+10]; its c-th column carries SEL[q, c] at row q*LBLK + i.
    """
    sel = np.zeros((NSIG, NSIG), dtype=np.float32)
    for c in range(M):
        sel[c, c] = 1.0
    sel[9, 8] = -2.0
    sel[8, 9] = 2.0
    out = np.zeros((BW, BW), dtype=np.float32)
    for i in range(LBLK):
        for q in range(NSIG):
            for c in range(NSIG):
                out[q * LBLK + i, i * NSIG + c] = sel[q, c]
    return out


def _cst2() -> np.ndarray:
    c = np.zeros((NSIG, C2W), dtype=np.float32)
    a1 = _assignment_matrix()
    c[0:M, RHS4_C : RHS4_C + K] = a1
    c[0:M, RHS4_C + K : RHS4_C + 2 * K] = a1
    c[8, RHS4_C : RHS4_C + K] = 1.0
    c[9, RHS4_C + K : RHS4_C + 2 * K] = 1.0
    c[0:M, PA2_C : PA2_C + K] = a1
    c[8, PA2_C : PA2_C + K] = -2.0
    c[0 : M + 1, ONES8_C] = 1.0
    c[0:M, WZ_C] = 1.0
    c[9, WZ_C] = 1.0
    c[9, HE9_C] = 0.5
    c[8, MHE8_C] = -0.5
    return c


def _interleave(sample: np.ndarray) -> np.ndarray:
    """[NSIG, T] -> [P, NSIG*NCHUNK]: R[p, b*100+s*10+i] = D[s, p*250+b*10+i]."""
    v = sample.reshape(NSIG, P, NBLK, LBLK).transpose(1, 2, 0, 3)
    return np.ascontiguousarray(v).reshape(P, NSIG * NCHUNK)


def _build_kernel() -> bass.Bass:
    nc = bass.Bass(trn_type="TRN2")
    data = nc.declare_dram_parameter("data", [P, NSIG * NCHUNK], F32, isOutput=False)
    cst1 = nc.declare_dram_parameter("cst1", [BW, BW], F32, isOutput=False)
    cst2 = nc.declare_dram_parameter("cst2", [NSIG, C2W], F32, isOutput=False)
    out = nc.declare_dram_parameter("loss", [1, 1], F32, isOutput=True)

    with ExitStack() as ctx:
        sb = lambda name, shape, dt=F32: ctx.enter_context(
            nc.sbuf_tensor(name, shape, dt)
        )
        ps = lambda name, shape: ctx.enter_context(nc.psum_tensor(name, shape, F32))

        nat = sb("nat", [P, NSIG * NCHUNK])
        rint = sb("rint", [P, NSIG * NCHUNK], BF16)
        junk = sb("junk", [P, 1])
        sentd = sb("sentd", [P, 1])
        sel = sb("sel", [BW, BW])
        csb2 = sb("csb2", [NSIG, C2W])
        cb2 = sb("cb2", [NSIG, C2W], BF16)
        pc = sb("pc", [BW, BW])
        zq = sb("zq", [NSIG, NSIG + 1], BF16)
        scs = sb("scs", [1, 40])
        eev = sb("eev", [1, 1])
        recipv = sb("recipv", [1, 1])
        red8 = sb("red8", [1, 1])
        s0s = sb("s0s", [1, 1])
        e1s = sb("e1s", [1, 1])
        prod4 = sb("prod4", [M + 1, 2 * K], BF16)
        d0sb = sb("d0sb", [1, K])
        pk = sb("pk", [1, K])
        mn = sb("mn", [1, 1])
        lg = sb("lg", [1, 1])

        gp = ps("gp", [BW, BW])
        qz10 = ps("qz10", [NSIG, NSIG])
        qt4 = ps("qt4", [M + 1, 2 * K])
        ne = ps("ne", [1, 2 * K])
        scp = ps("scp", [1, 40])

        dsem_w = [
            ctx.enter_context(nc.semaphore(f"dsem_w{w}")) for w in range(N_WAVES)
        ]
        dsem_c1 = ctx.enter_context(nc.semaphore("dsem_c1"))
        dsem_c2 = ctx.enter_context(nc.semaphore("dsem_c2"))
        pe_sem = ctx.enter_context(nc.semaphore("pe_sem"))
        dve_sem = ctx.enter_context(nc.semaphore("dve_sem"))
        act_sem = ctx.enter_context(nc.semaphore("act_sem"))
        gp_sem = ctx.enter_context(nc.semaphore("gp_sem"))
        dout = ctx.enter_context(nc.semaphore("dout"))
        waste = ctx.enter_context(nc.semaphore("waste"))

        # ---------------- SP: data waves + sentinels ------------------------
        for w in range(N_WAVES):
            c0, c1 = WAVE_EDGES[w] * BW, WAVE_EDGES[w + 1] * BW
            nc.sync.dma_start(out=nat[:, c0:c1], in_=data[:, c0:c1]).then_inc(
                waste, 16
            )
            nc.sync.dma_start(out=sentd[:, :], in_=junk[:, :]).then_inc(dsem_w[w], 16)

        # ---------------- GPSIMD: eev only ----------------------------------
        nc.gpsimd.wait_ge(act_sem, 2)
        nc.gpsimd.tensor_mul(
            eev[:, :], scs[0:1, 20:21], scs[0:1, 32:33]
        ).then_inc(gp_sem, 1)

        # ---------------- DVE ------------------------------------------------
        for ci in (0, 1, 2):  # 1..3: wave-0 cast chunks
            c0, c1 = CAST_EDGES[ci] * BW, CAST_EDGES[ci + 1] * BW
            nc.vector.wait_ge(dsem_w[0], 16)
            nc.vector.tensor_copy(rint[:, c0:c1], nat[:, c0:c1]).then_inc(dve_sem, 1)
        nc.vector.wait_ge(dsem_w[1], 16)
        for ci in (3, 4):  # 4, 5: wave-1 cast chunks
            c0, c1 = CAST_EDGES[ci] * BW, CAST_EDGES[ci + 1] * BW
            nc.vector.tensor_copy(rint[:, c0:c1], nat[:, c0:c1]).then_inc(dve_sem, 1)
        nc.vector.wait_ge(dsem_c2, 16)
        nc.vector.tensor_copy(cb2[:, :], csb2[:, :]).then_inc(dve_sem, 1)  # 6
        nc.vector.wait_ge(pe_sem, 5)
        nc.vector.tensor_copy(pc[:, :], gp[:, :]).then_inc(dve_sem, 1)  # 7
        nc.vector.wait_ge(pe_sem, 6)
        nc.vector.tensor_copy(zq[:, 0:M], qz10[:, 0:M]).then_inc(dve_sem, 1)  # 8
        nc.vector.tensor_copy(
            zq[:, NSIG - 1 : NSIG + 1], qz10[:, M:NSIG]
        ).then_inc(dve_sem, 1)  # 9
        with nc.allow_low_precision("rs row-sums to bf16: |G8| ~ 3e4, 0.4% ok"):
            nc.vector.tensor_reduce(
                zq[:, M : M + 1], qz10[:, 0:M], axis=mybir.AxisListType.X,
                op=mybir.AluOpType.add,
            ).then_inc(dve_sem, 1)  # 10
        nc.vector.wait_ge(pe_sem, 7)
        nc.vector.tensor_tensor(
            prod4[0:M, 0:K], qt4[0:M, 0:K], cb2[0:M, RHS4_C : RHS4_C + K],
            op=mybir.AluOpType.mult,
        ).then_inc(dve_sem, 1)  # 11
        nc.vector.wait_ge(pe_sem, 8)
        nc.vector.tensor_tensor(
            prod4[0 : M + 1, K : 2 * K], qt4[0 : M + 1, K : 2 * K],
            cb2[0 : M + 1, PA2_C : PA2_C + K],
            op=mybir.AluOpType.mult,
        ).then_inc(dve_sem, 1)  # 12
        nc.vector.wait_ge(gp_sem, 1)
        nc.vector.reciprocal(recipv[:, :], eev[:, :]).then_inc(dve_sem, 1)  # 13
        # s0s = (1+tau)*E0 + red8 (act>=2 implied via gp_sem path)
        nc.vector.scalar_tensor_tensor(
            s0s[:, :], scs[0:1, 20:21], 1.0 + TAU, red8[:, :],
            op0=mybir.AluOpType.mult, op1=mybir.AluOpType.add,
        ).then_inc(dve_sem, 1)  # 14
        nc.vector.wait_ge(act_sem, 3)
        nc.vector.wait_ge(pe_sem, 11)
        nc.vector.tensor_scalar_add(
            d0sb[:, :], ne[0:1, K : 2 * K], s0s[0:1, 0:1]
        ).then_inc(dve_sem, 1)  # 15
        nc.vector.wait_ge(dve_sem, 15)
        nc.vector.scalar_tensor_tensor(
            pk[:, :], ne[0:1, 0:K], e1s[0:1, 0:1], d0sb[:, :],
            op0=mybir.AluOpType.add, op1=mybir.AluOpType.mult,
        ).then_inc(dve_sem, 1)  # 16
        nc.vector.wait_ge(dve_sem, 16)
        nc.vector.tensor_reduce(
            mn[:, :], pk[:, :], axis=mybir.AxisListType.X, op=mybir.AluOpType.min
        ).then_inc(dve_sem, 1)  # 17

        # ---------------- ACT: consts DMA, scalar chain, ln, output ---------
        nc.scalar.dma_start(out=csb2[:, :], in_=cst2[:, :]).then_inc(dsem_c2, 16)
        nc.scalar.dma_start(out=sel[:, :], in_=cst1[:, :]).then_inc(dsem_c1, 16)
        nc.scalar.wait_ge(dsem_c2, 16)
        nc.scalar.activation(  # Ln table preload (dummy)
            scs[0:1, 30:31], csb2[0:1, 0:1], mybir.ActivationFunctionType.Ln
        )
        nc.scalar.wait_ge(pe_sem, 10)
        nc.scalar.activation(
            scs[0:1, 8:33], scp[0:1, 8:33], mybir.ActivationFunctionType.Copy
        ).then_inc(act_sem, 1)
        # red8 = sum(scp[0:8]) = sum(G8) + 2*sum(C0), via accum_out
        nc.scalar.activation(
            scs[0:1, 0:8], scp[0:1, 0:8], mybir.ActivationFunctionType.Copy,
            accum_out=red8[:, :],
        ).then_inc(act_sem, 1)
        nc.scalar.wait_ge(act_sem, 2)
        nc.scalar.activation(
            e1s[:, :], scs[0:1, 32:33], mybir.ActivationFunctionType.Copy,
            scale=1.0 + TAU,
        ).then_inc(act_sem, 1)
        # lg = ln(mn * (1/(E0*E1))); host multiplies by 10/ln10
        nc.scalar.wait_ge(dve_sem, 17)
        nc.scalar.activation(
            lg[:, :], mn[:, :], mybir.ActivationFunctionType.Ln, scale=recipv[0:1, 0:1]
        )
        nc.scalar.dma_start(out=out[:, :], in_=lg[:, :]).then_inc(dout, 16)

        # ---------------- PE -------------------------------------------------
        for ci in range(len(CAST_EDGES) - 1):
            b0, b1 = CAST_EDGES[ci], CAST_EDGES[ci + 1]
            nc.tensor.wait_ge(dve_sem, ci + 1)
            for blk in range(b0, b1):
                cols = rint[:, blk * BW : (blk + 1) * BW]
                mm = nc.tensor.matmul(
                    gp[:, :], cols, cols, start=(blk == 0), stop=(blk == NBLK - 1)
                )
            mm.then_inc(pe_sem, 1)  # 1..5
        nc.tensor.wait_ge(dsem_c1, 16)
        nc.tensor.wait_ge(dve_sem, 7)  # pc copied
        for i in range(LBLK):
            mm = nc.tensor.matmul(
                qz10[:, :],
                sel[:, i * NSIG : (i + 1) * NSIG],
                pc[:, i::LBLK],
                start=(i == 0),
                stop=(i == LBLK - 1),
            )
        mm.then_inc(pe_sem, 1)  # 6
        nc.tensor.wait_ge(dve_sem, 10)  # zq assembled (cb2 at 6 also covered)
        nc.tensor.matmul(
            qt4[0:M, 0:K], zq[0:NSIG, 0:M], cb2[0:NSIG, RHS4_C : RHS4_C + K]
        ).then_inc(pe_sem, 1)  # 7
        nc.tensor.matmul(
            qt4[0 : M + 1, K : 2 * K], zq[0:NSIG, 0 : M + 1],
            cb2[0:NSIG, RHS4_C + K : RHS4_C + 2 * K],
        ).then_inc(pe_sem, 1)  # 8
        nc.tensor.matmul(
            scp[0:1, 0:11], cb2[0:NSIG, WZ_C : WZ_C + 1], zq[:, :],
        )
        nc.tensor.matmul(
            scp[0:1, 11:22], cb2[0:NSIG, HE9_C : HE9_C + 1], zq[:, :],
        )
        nc.tensor.matmul(
            scp[0:1, 22:33], cb2[0:NSIG, MHE8_C : MHE8_C + 1], zq[:, :],
        ).then_inc(pe_sem, 2)  # 10
        nc.tensor.wait_ge(dve_sem, 11)  # prod4 first half
        nc.tensor.matmul(
            ne[0:1, 0:K], cb2[0:M, ONES8_C : ONES8_C + 1], prod4[0:M, 0:K],
            start=True, stop=True,
        )
        nc.tensor.wait_ge(dve_sem, 12)  # prod4b9
        nc.tensor.matmul(
            ne[0:1, K : 2 * K], cb2[0 : M + 1, ONES8_C : ONES8_C + 1],
            prod4[0 : M + 1, K : 2 * K], start=True, stop=True,
        ).then_inc(pe_sem, 1)  # 11

    return nc


_NC_CACHE: bass.Bass | None = None


def kernel(estimated_sources: np.ndarray, input_mixtures: np.ndarray) -> np.ndarray:
    global _NC_CACHE
    assert estimated_sources.shape == (B, M, T)
    assert input_mixtures.shape == (B, NMIX, T)
    if _NC_CACHE is None:
        _NC_CACHE = _build_kernel()
    nc = _NC_CACHE

    cst1 = _sel_diag()
    cst2 = _cst2()
    est = np.asarray(estimated_sources, dtype=np.float32)
    mx = np.asarray(input_mixtures, dtype=np.float32)
    in_maps = [
        {
            "data": _interleave(np.concatenate([est[b], mx[b]], axis=0)),
            "cst1": cst1,
            "cst2": cst2,
        }
        for b in range(B)
    ]
    res = run_bass_kernel_spmd(nc, in_maps, core_ids=list(range(B)))
    vals = np.array([res.results[b]["loss"][0, 0] for b in range(B)], dtype=np.float32)
    return np.asarray(np.float32(LOG10_SCALE) * vals.mean(), dtype=np.float32)
